# revision 1
# baseline (speedup 1.0000x reference)
"""MultiHeadAttention TRN2 kernel.

Full inputs -> shard over 8 NeuronCores -> full outputs (context, avg_attn).

Sharding: core = (batch b, query-half qh).  Each core computes its own
Q/K/V projections (K/V duplicated between the two cores of a batch),
attention for all 16 heads over its 512-query shard, the output
projection, and the head-averaged attention map.

Host-side preprocessing:
  - key_padding_mask compaction: only the unmasked key/value rows are
    shipped (padded to a fixed S_PAD=640 >> max unmasked count), which
    halves the S-dimension work.  Masked positions of avg_attn are
    exactly 0 in the reference (exp(-1e9) == 0 in fp32), so the host
    scatters the compact result back and leaves zeros elsewhere.
  - inputs/weights are transposed + cast to fp16 (PE runs fp16 at full
    rate with fp32 PSUM accumulation); per-partition bias table and
    per-head aug rows are pre-replicated so each lands in ONE DMA.

Device dataflow (per core).  Everything on the attention path lives in
the TRANSPOSED orientation [s, q] so attn@V needs no transposes at all;
the host untransposes ctxT / avgT when reassembling.
  qT = WqT.T @ xqT  (+bq per-partition)  -> q_aug big tile [65, h, q]
  kT = WkT.T @ xkT  (+bk)                -> k_aug big tile [65, h, s]
  v  = xvT.T @ WvT  (+valid01 x bv rank-1) -> [s, h] tiles
  scoresT[s,q] = k_aug.T @ q_aug  (K=65: 65th row adds -30000 on pads)
  A = exp(scale*scoresT)                  (ACT, per s-tile)
  r[q] = ones.T @ A                       (K=128 ones-column matvecs, PE)
  invr_bc = ones x (1/r)                  (K=1 f16 matmul + ACT copy)
  An = A * invr_bc (DVE per s-tile); avg_accT += An (one merged DVE add)
  outT[d,q] = sum_s v[s,d] * An[s,q]      (col-group-packed head pairs)
  ctxT = WoT.T @ outT (+bo)
"""

import sys

if "/opt/trn_rl_repo" not in sys.path:
    sys.path.insert(0, "/opt/trn_rl_repo")

from contextlib import ExitStack

import numpy as np

import concourse.bass as bass
import concourse.mybir as mybir
import concourse.tile as tile
from concourse import bacc
from concourse.bass_utils import run_bass_kernel_spmd

F16 = mybir.dt.float16
F32 = mybir.dt.float32

B, Q, S, H, NH = 4, 1024, 1024, 1024, 16
HD = H // NH  # 64
QSH = Q // 2  # per-core query shard
S_PAD = 640  # compacted+padded key length (P[Binom(1024,.5) > 640] ~ 0)
N_CORES = 8
SCALE = 1.0 / float(np.sqrt(HD))
MASK_NEG = -30000.0  # exp(SCALE*(x + MASK_NEG)) == 0 exactly in fp32


def build_nc(h=H, qsh=QSH, s_pad=S_PAD, reps=1):
    nh = h // HD
    kt_n = h // 128  # contraction tiles
    ot_n = h // 128  # output-feature tiles
    st_n = s_pad // 128
    sq = st_n * qsh  # merged free size of one head's A / avg tiles

    nc = bacc.Bacc("TRN2", target_bir_lowering=False)

    xqT = nc.dram_tensor("xqT", [h, qsh], F16, kind="ExternalInput")
    xkT = nc.dram_tensor("xkT", [h, s_pad], F16, kind="ExternalInput")
    xvT = nc.dram_tensor("xvT", [h, s_pad], F16, kind="ExternalInput")
    wqT = nc.dram_tensor("wqT", [h, h], F16, kind="ExternalInput")
    wkT = nc.dram_tensor("wkT", [h, h], F16, kind="ExternalInput")
    wvT = nc.dram_tensor("wvT", [h, h], F16, kind="ExternalInput")
    woT = nc.dram_tensor("woT", [h, h], F16, kind="ExternalInput")
    btab_d = nc.dram_tensor("btab", [128, 3 * ot_n], F32, kind="ExternalInput")
    bvr_d = nc.dram_tensor("bvr", [1, h], F16, kind="ExternalInput")
    val_d = nc.dram_tensor("val01", [1, s_pad], F16, kind="ExternalInput")
    mkbrep_d = nc.dram_tensor("mkbrep", [1, nh * s_pad], F16,
                              kind="ExternalInput")
    onesrep_d = nc.dram_tensor("onesrep", [1, nh * qsh], F16,
                               kind="ExternalInput")
    ctxT = nc.dram_tensor("ctxT", [h, qsh], F32, kind="ExternalOutput")
    avgT = nc.dram_tensor("avgT", [s_pad, qsh], F32, kind="ExternalOutput")

    with tile.TileContext(nc) as tc, ExitStack() as ctx:
        pw = ctx.enter_context(tc.tile_pool(name="w", bufs=2))
        px = ctx.enter_context(tc.tile_pool(name="x", bufs=2))
        pqa = ctx.enter_context(tc.tile_pool(name="qaug", bufs=1))
        pka = ctx.enter_context(tc.tile_pool(name="kaug", bufs=1))
        pv = ctx.enter_context(tc.tile_pool(name="vsb", bufs=st_n))
        pout = ctx.enter_context(tc.tile_pool(name="outT", bufs=ot_n))
        pavg = ctx.enter_context(tc.tile_pool(name="avga", bufs=1))
        pA = ctx.enter_context(tc.tile_pool(name="A", bufs=2))
        pAn = ctx.enter_context(tc.tile_pool(name="An", bufs=3))
        pbc = ctx.enter_context(tc.tile_pool(name="bc", bufs=3))
        pr = ctx.enter_context(tc.tile_pool(name="r", bufs=4))
        pstg = ctx.enter_context(tc.tile_pool(name="stg", bufs=2))
        prow = ctx.enter_context(tc.tile_pool(name="row", bufs=1))
        pctx = ctx.enter_context(tc.tile_pool(name="ctxe", bufs=2))
        pps = ctx.enter_context(tc.tile_pool(name="psp", bufs=2, space="PSUM"))
        psc = ctx.enter_context(tc.tile_pool(name="pssc", bufs=2, space="PSUM"))
        ppa = ctx.enter_context(tc.tile_pool(name="psav", bufs=2, space="PSUM"))
        prb = ctx.enter_context(tc.tile_pool(name="psrb", bufs=2, space="PSUM"))

        for _rep in range(reps):
            # ---- constants: one DMA each ----
            bvr = prow.tile([1, h], F16, tag="bvr")
            nc.sync.dma_start(bvr[:], bvr_d[:])
            val01 = prow.tile([1, s_pad], F16, tag="val")
            nc.sync.dma_start(val01[:], val_d[:])
            onescol = prow.tile([128, 1], F16, tag="onescol")
            nc.vector.memset(onescol[:], 1.0)
            onesr16 = prow.tile([1, 128], F16, tag="onesr16")
            nc.vector.memset(onesr16[:], 1.0)
            btab = prow.tile([128, 3 * ot_n], F32, tag="btab")
            nc.sync.dma_start(btab[:], btab_d[:])

            def bq_t(ot):
                return btab[:, ot:ot + 1]

            def bk_t(ot):
                return btab[:, ot_n + ot:ot_n + ot + 1]

            def bo_t(ot):
                return btab[:, 2 * ot_n + ot:2 * ot_n + ot + 1]

            # ---- q projection -> q_aug big tile [65., nh, qsh] ----
            xq = px.tile([128, kt_n, qsh], F16, tag="xin", name="xq")
            nc.sync.dma_start(xq[:], xqT.rearrange("(t p) c -> p t c", p=128))
            wq = pw.tile([128, kt_n, h], F16, tag="w", name="wq")
            nc.sync.dma_start(wq[:], wqT.rearrange("(t p) c -> p t c", p=128))
            qbig = pqa.tile([128, nh, qsh], F16, tag="qaug", name="qaug")
            nc.sync.dma_start(
                qbig[64:65, :, :],
                onesrep_d[:].rearrange("o (n c) -> o n c", n=nh))
            for ot in range(ot_n):
                ps = pps.tile([128, qsh], F32, tag="psp")
                for kt in range(kt_n):
                    nc.tensor.matmul(
                        ps[:], wq[:, kt, ot * 128:(ot + 1) * 128],
                        xq[:, kt, :], start=(kt == 0), stop=(kt == kt_n - 1))
                he, ho = 2 * ot, 2 * ot + 1
                nc.vector.tensor_scalar_add(qbig[0:64, he, :], ps[0:64, :],
                                            bq_t(ot)[0:64, :])
                stg = pstg.tile([128, s_pad], F16, tag="stg")
                nc.vector.tensor_scalar_add(stg[64:128, 0:qsh], ps[64:128, :],
                                            bq_t(ot)[64:128, :])
                nc.sync.dma_start(qbig[0:64, ho, :], stg[64:128, 0:qsh])

            # ---- k projection -> k_aug big tile [65., nh, s_pad] ----
            xk = px.tile([128, kt_n, s_pad], F16, tag="xin", name="xk")
            nc.sync.dma_start(xk[:], xkT.rearrange("(t p) c -> p t c", p=128))
            wk = pw.tile([128, kt_n, h], F16, tag="w", name="wk")
            nc.sync.dma_start(wk[:], wkT.rearrange("(t p) c -> p t c", p=128))
            kbig = pka.tile([128, nh, s_pad], F16, tag="kaug", name="kaug")
            nc.sync.dma_start(
                kbig[64:65, :, :],
                mkbrep_d[:].rearrange("o (n c) -> o n c", n=nh))
            for ot in range(ot_n):
                he, ho = 2 * ot, 2 * ot + 1
                stg = pstg.tile([128, s_pad], F16, tag="stg")
                for c0 in range(0, s_pad, 512):
                    c1 = min(c0 + 512, s_pad)
                    ps = pps.tile([128, qsh], F32, tag="psp", name="psk")
                    for kt in range(kt_n):
                        nc.tensor.matmul(
                            ps[:, 0:c1 - c0],
                            wk[:, kt, ot * 128:(ot + 1) * 128],
                            xk[:, kt, c0:c1],
                            start=(kt == 0), stop=(kt == kt_n - 1))
                    nc.vector.tensor_scalar_add(kbig[0:64, he, c0:c1],
                                                ps[0:64, 0:c1 - c0],
                                                bk_t(ot)[0:64, :])
                    nc.vector.tensor_scalar_add(stg[64:128, c0:c1],
                                                ps[64:128, 0:c1 - c0],
                                                bk_t(ot)[64:128, :])
                nc.sync.dma_start(kbig[0:64, ho, :], stg[64:128, :])

            # ---- v projection -> v_sb [s, h] tiles ----
            xv = px.tile([128, kt_n, s_pad], F16, tag="xin", name="xv")
            nc.sync.dma_start(xv[:], xvT.rearrange("(t p) c -> p t c", p=128))
            wv = pw.tile([128, kt_n, h], F16, tag="w", name="wv")
            nc.sync.dma_start(wv[:], wvT.rearrange("(t p) c -> p t c", p=128))
            v_sb = [pv.tile([128, h], F16, tag="vsb", name="vsb")
                    for _ in range(st_n)]
            for st in range(st_n):
                for c0 in range(0, h, 512):
                    c1 = min(c0 + 512, h)
                    ps = pps.tile([128, qsh], F32, tag="psp")
                    for kt in range(kt_n):
                        nc.tensor.matmul(
                            ps[:, 0:c1 - c0], xv[:, kt, st * 128:(st + 1) * 128],
                            wv[:, kt, c0:c1], start=(kt == 0), stop=False)
                    nc.tensor.matmul(ps[:, 0:c1 - c0],
                                     val01[0:1, st * 128:(st + 1) * 128],
                                     bvr[0:1, c0:c1], start=False, stop=True)
                    nc.scalar.copy(v_sb[st][:, c0:c1], ps[:, 0:c1 - c0])

            # ---- attention, fully transposed [s, q] ----
            avg_acc = pavg.tile([128, sq], F16, tag="avga", name="avga")
            nc.vector.memset(avg_acc[:], 0.0)
            out_sb = [pout.tile([128, qsh], F16, tag="outT", name="outT")
                      for _ in range(ot_n)]
            an_pair = [None, None]
            for hd in range(nh):
                a_big = pA.tile([128, st_n, qsh], F16, tag="A", name="A")
                for st in range(st_n):
                    ps = psc.tile([128, qsh], F32, tag="pssc")
                    nc.tensor.matmul(
                        ps[:], kbig[0:65, hd, st * 128:(st + 1) * 128],
                        qbig[0:65, hd, :], start=True, stop=True)
                    nc.scalar.activation(a_big[:, st, :], ps[:],
                                         mybir.ActivationFunctionType.Exp,
                                         scale=SCALE)
                # r[q] = sum_s A[s,q] via ones-column matvec
                psr = prb.tile([128, qsh], F32, tag="psrb", name="psr")
                for st in range(st_n):
                    nc.tensor.matmul(psr[0:1, :], onescol[:], a_big[:, st, :],
                                     start=(st == 0), stop=(st == st_n - 1))
                invr = pr.tile([1, qsh], F16, tag="invr")
                with nc.allow_low_precision(reason="1/r broadcast in f16"):
                    nc.vector.reciprocal(invr[:], psr[0:1, :])
                # broadcast 1/r to 128 partitions: ones[1,128].T @ invr[1,q]
                psb = prb.tile([128, qsh], F32, tag="psrb", name="psb")
                nc.tensor.matmul(psb[:], onesr16[:], invr[:],
                                 start=True, stop=True)
                invr_bc = pbc.tile([128, qsh], F16, tag="bc", name="bc")
                nc.scalar.copy(invr_bc[:], psb[:])
                # normalize (per s-tile) + one merged avg accumulate
                an_big = pAn.tile([128, st_n, qsh], F16, tag="An", name="An")
                for st in range(st_n):
                    nc.vector.tensor_tensor(an_big[:, st, :], a_big[:, st, :],
                                            invr_bc[:],
                                            op=mybir.AluOpType.mult)
                nc.vector.tensor_tensor(
                    avg_acc[:], avg_acc[:],
                    an_big[:].rearrange("p a b -> p (a b)"),
                    op=mybir.AluOpType.add)
                an_pair[hd % 2] = an_big
                if hd % 2 == 1:
                    he, ho = hd - 1, hd
                    pse = ppa.tile([128, qsh], F32, tag="psav", name="pse")
                    pso_ = ppa.tile([128, qsh], F32, tag="psav", name="pso_")
                    for st in range(st_n):
                        first, last = st == 0, st == st_n - 1
                        nc.tensor.matmul(
                            pse[0:64, :], v_sb[st][:, he * 64:(he + 1) * 64],
                            an_pair[0][:, st, :], start=first, stop=last,
                            tile_position=(0, 0))
                        nc.tensor.matmul(
                            pso_[64:128, :], v_sb[st][:, ho * 64:(ho + 1) * 64],
                            an_pair[1][:, st, :], start=first, stop=last,
                            tile_position=(0, 64))
                    nc.scalar.copy(out_sb[hd // 2][0:64, :], pse[0:64, :])
                    nc.scalar.copy(out_sb[hd // 2][64:128, :], pso_[64:128, :])

            # ---- output projection (ctx big tile -> one DMA) ----
            wo = pw.tile([128, kt_n, h], F16, tag="w", name="wo")
            nc.sync.dma_start(wo[:], woT.rearrange("(t p) c -> p t c", p=128))
            for ot in range(ot_n):
                ps = pps.tile([128, qsh], F32, tag="psp")
                for kt in range(kt_n):
                    nc.tensor.matmul(
                        ps[:], wo[:, kt, ot * 128:(ot + 1) * 128], out_sb[kt][:],
                        start=(kt == 0), stop=(kt == kt_n - 1))
                ctx_e = pctx.tile([128, qsh], F32, tag="ctxe")
                nc.vector.tensor_scalar_add(ctx_e[:], ps[:], bo_t(ot)[:])
                nc.sync.dma_start(ctxT[ot * 128:(ot + 1) * 128, :], ctx_e[:])

            # ---- avg_attn finalize (transposed; host untransposes) ----
            af = pstg.tile([128, sq], F32, tag="avgf", name="avgf", bufs=1)
            nc.vector.tensor_scalar_mul(af[:], avg_acc[:], 1.0 / nh)
            nc.sync.dma_start(avgT.rearrange("(t p) c -> p t c", p=128),
                              af[:].rearrange("p (t c) -> p t c", t=st_n))

    nc.compile()
    return nc


_NC_CACHE = {}


def _get_nc():
    if "nc" not in _NC_CACHE:
        _NC_CACHE["nc"] = build_nc()
    return _NC_CACHE["nc"]


def make_in_maps(query, key, value, key_padding_mask,
                 Wq, bq, Wk, bk, Wv, bv, Wo, bo):
    query = np.asarray(query, np.float32)
    key = np.asarray(key, np.float32)
    value = np.asarray(value, np.float32)
    mask = np.asarray(key_padding_mask, bool)
    wqT16 = np.ascontiguousarray(np.asarray(Wq, np.float32).T.astype(np.float16))
    wkT16 = np.ascontiguousarray(np.asarray(Wk, np.float32).T.astype(np.float16))
    wvT16 = np.ascontiguousarray(np.asarray(Wv, np.float32).T.astype(np.float16))
    woT16 = np.ascontiguousarray(np.asarray(Wo, np.float32).T.astype(np.float16))
    ot_n = H // 128
    btab = np.empty((128, 3 * ot_n), np.float32)
    for i, b in enumerate((bq, bk, bo)):
        btab[:, i * ot_n:(i + 1) * ot_n] = \
            np.asarray(b, np.float32).reshape(ot_n, 128).T
    bvr = np.asarray(bv, np.float32).reshape(1, H).astype(np.float16)
    onesrep = np.ones((1, NH * QSH), np.float16)

    idx_list = [np.nonzero(~mask[b])[0] for b in range(B)]
    in_maps = []
    for core in range(N_CORES):
        b, qh = divmod(core, 2)
        idx = idx_list[b]
        se = len(idx)
        xq = query[b, qh * QSH:(qh + 1) * QSH, :]
        xk = np.zeros((S_PAD, H), np.float16)
        xk[:se] = key[b, idx, :].astype(np.float16)
        xv = np.zeros((S_PAD, H), np.float16)
        xv[:se] = value[b, idx, :].astype(np.float16)
        val01 = np.zeros((1, S_PAD), np.float16)
        val01[0, :se] = 1.0
        mkb = np.full((S_PAD,), MASK_NEG, np.float16)
        mkb[:se] = 0.0
        in_maps.append({
            "xqT": np.ascontiguousarray(xq.T.astype(np.float16)),
            "xkT": np.ascontiguousarray(xk.T),
            "xvT": np.ascontiguousarray(xv.T),
            "wqT": wqT16, "wkT": wkT16, "wvT": wvT16, "woT": woT16,
            "btab": btab, "bvr": bvr, "val01": val01,
            "mkbrep": np.tile(mkb, NH).reshape(1, -1),
            "onesrep": onesrep,
        })
    return in_maps, idx_list


def assemble(results, idx_list):
    context = np.empty((B, Q, H), np.float32)
    avg = np.zeros((B, Q, S), np.float32)
    for core in range(N_CORES):
        b, qh = divmod(core, 2)
        rows = slice(qh * QSH, (qh + 1) * QSH)
        context[b, rows, :] = results[core]["ctxT"].T
        idx = idx_list[b]
        # advanced-index dims (b, idx) move to the front; avgT is [s, q]
        avg[b, rows, idx] = results[core]["avgT"][:len(idx), :]
    return context, avg


def _numpy_fallback(query, key, value, key_padding_mask,
                    Wq, bq, Wk, bk, Wv, bv, Wo, bo):
    """Exact fp32 reference path, used only if the mask compaction budget
    would overflow (cannot happen for the spec'd input distribution)."""
    q = (query @ Wq.T + bq).reshape(B, Q, NH, HD).transpose(0, 2, 1, 3)
    k = (key @ Wk.T + bk).reshape(B, S, NH, HD).transpose(0, 2, 1, 3)
    v = (value @ Wv.T + bv).reshape(B, S, NH, HD).transpose(0, 2, 1, 3)
    s = np.einsum("bhqd,bhsd->bhqs", q, k) / np.sqrt(HD)
    s = np.where(np.asarray(key_padding_mask, bool)[:, None, None, :], -1e9, s)
    s = s - s.max(-1, keepdims=True)
    a = np.exp(s)
    a /= a.sum(-1, keepdims=True)
    out = np.einsum("bhqs,bhsd->bhqd", a, v)
    out = out.transpose(0, 2, 1, 3).reshape(B, Q, H)
    return (out @ Wo.T + bo).astype(np.float32), \
        a.mean(axis=1).astype(np.float32)


def kernel(query, key, value, key_padding_mask,
           Wq, bq, Wk, bk, Wv, bv, Wo, bo):
    assert query.shape == (B, Q, H) and key.shape == (B, S, H)
    mask = np.asarray(key_padding_mask, bool)
    if max((~mask[b]).sum() for b in range(B)) > S_PAD:
        return _numpy_fallback(query, key, value, key_padding_mask,
                               Wq, bq, Wk, bk, Wv, bv, Wo, bo)
    in_maps, idx_list = make_in_maps(query, key, value, key_padding_mask,
                                     Wq, bq, Wk, bk, Wv, bv, Wo, bo)
    res = run_bass_kernel_spmd(_get_nc(), in_maps,
                               core_ids=list(range(N_CORES)))
    return assemble(res.results, idx_list)

